# revision 43
# baseline (speedup 1.0000x reference)
"""Trainium2 Bass kernel for nn_LocalMixtureNN (self-contained).

Strategy
--------
Pure data parallel over batch: 8 cores x 4 batches. Within a core the 128
(s, b) positions live on the 128 SBUF partitions (pos = s*4 + b, s-major so
n-gram window shifts are partition shifts by 4*k, realized as matmuls with
constant shift matrices).

The LSTM recurrence runs hidden-major: h/c state is (128 hid, 4 batch).
Sigmoid is folded into tanh (sigma(z) = (tanh(z/2)+1)/2) with all the 0.5
factors pre-folded into host-prepared weights, and doubled cell/hidden state
(C2 = 2c, H2 = 2h), so every activation in the whole kernel comes from the
single "exp_and_others" ACT table set (tanh + exp + square) plus one early
"trig" set load for sin/cos. All sqrt/rsqrt are division-free Newton
iterations on prescaled inputs (no sqrt table load), and every normalization
(1/(n0+eps), 1/(n1+eps), measurement-kernel 1/kn) is factored out of the
dependency spine and applied late as per-partition scales (on M per position,
on featT per unit), so the Newton chains hide behind the measurement matmuls.

The density-matrix measurement collapses algebraically: m[p,u] =
sum_k ww_k |v_u^H x_{p+k}|^2 with |v^H x|^2 = (P+T)^2 + (R-Q)^2 where
P,Q,R,T are four real matmuls (real/imag vectors x meas_r/meas_i^T).

Weights are shipped in two packed (128, X) DMAs to avoid per-tensor DMA
issue serialization.
"""

import numpy as np

try:
    import concourse  # noqa: F401
except ImportError:  # pragma: no cover
    import sys

    sys.path.insert(0, "/opt/trn_rl_repo")

from contextlib import ExitStack

import concourse.bass as bass
import concourse.bacc as bacc
import concourse.tile as tile
import concourse.mybir as mybir

dt = mybir.dt
F32 = dt.float32
R32 = dt.float32r
BF16 = dt.bfloat16
I32 = dt.int32
AF = mybir.ActivationFunctionType
OP = mybir.AluOpType

NCORES = 8
B, S = 32, 32
BL = B // NCORES  # 4 batches per core
NPOS = S * BL  # 128 positions per core, pos = s*BL + b
VOCAB, EMBD, AUDIO, HID = 5000, 300, 74, 128
D1 = 16
D2 = 16
DIM = D1 * D2  # 256
UNITS, CELL = 128, 64
HALF_PI = float(np.pi / 2)
SQRT8 = float(np.sqrt(8.0))
INV_SQRT8 = float(1.0 / np.sqrt(8.0))

# pack0: hot small fp32 constants (identity/MLP weights).
PACK0 = {}
_c = 0
for _name, _ncol in (
    ("ident", 128), ("fw1t", 64), ("fw2t", 64),
    ("w1b", 16), ("w2b", 16), ("w3b", 16),
    ("fb1", 1), ("fb2", 1), ("fw3t", 1), ("fb3", 1),
):
    PACK0[_name] = (_c, _ncol)
    _c += _ncol
PACK0_COLS = _c  # 308
# pack2: measurement kernel (fp32, needed late)
PACK2 = {}
_c = 0
for _name, _ncol in (("mr", 256), ("mi", 256)):
    PACK2[_name] = (_c, _ncol)
    _c += _ncol
PACK2_COLS = _c  # 512
# packB: bf16 weights (identity for bf16 matmuls, LSTM recurrent, amp0 proj,
# n-gram shift matrices)
PACKB = {}
_c = 0
for _name, _ncol in (
    ("identb", 128), ("whht", 512), ("wlinb", 16), ("s1tb", 128), ("s2tb", 128),
):
    PACKB[_name] = (_c, _ncol)
    _c += _ncol
PACKB_COLS = _c  # 912


def _newton_rsqrt(nc, pool, x_ap, iters, tag, eng=None):
    """r ~= rsqrt(x) for x (128,1), seed 2/(1+x), division-free Newton.

    eng selects the elementwise engine (nc.vector or nc.gpsimd); the seed
    reciprocal is DVE-only either way. Pool-engine safe (tensor_scalar /
    tensor_tensor only).
    """
    if eng is None:
        eng = nc.vector
    t = pool.tile([128, 1], F32, tag=f"{tag}_t")
    eng.tensor_scalar(out=t[:], in0=x_ap, scalar1=1.0, scalar2=None, op0=OP.add)
    r = pool.tile([128, 1], F32, tag=f"{tag}_r0")
    nc.vector.reciprocal(out=r[:], in_=t[:])
    r2 = pool.tile([128, 1], F32, tag=f"{tag}_r2")
    eng.tensor_scalar(out=r2[:], in0=r[:], scalar1=2.0, scalar2=None, op0=OP.mult)
    sneg = pool.tile([128, 1], F32, tag=f"{tag}_s")
    eng.tensor_scalar(
        out=sneg[:], in0=x_ap, scalar1=-0.5, scalar2=None, op0=OP.mult
    )
    cur = r2
    for _ in range(iters):
        rr = pool.tile([128, 1], F32, tag=f"{tag}_rr")
        eng.tensor_tensor(out=rr[:], in0=cur[:], in1=cur[:], op=OP.mult)
        z = pool.tile([128, 1], F32, tag=f"{tag}_z")
        eng.tensor_scalar(
            out=z[:], in0=rr[:], scalar1=sneg[:, 0:1], scalar2=1.5,
            op0=OP.mult, op1=OP.add,
        )
        nxt = pool.tile([128, 1], F32, tag=f"{tag}_rn")
        eng.tensor_tensor(out=nxt[:], in0=cur[:], in1=z[:], op=OP.mult)
        cur = nxt
    return cur


def _newton_rsqrt_q(nc, pool, x_ap, iters, tag, eng=None):
    """rsqrt via newton with quadratic minimax seed on x in [0.15, 2.6]."""
    if eng is None:
        eng = nc.vector
    C2, C1, C0 = 0.4290883, -1.76366757, 2.40807279
    t = pool.tile([128, 1], F32, tag=f"{tag}_t")
    eng.tensor_scalar(out=t[:], in0=x_ap, scalar1=C2, scalar2=C1,
                      op0=OP.mult, op1=OP.add)
    seed = pool.tile([128, 1], F32, tag=f"{tag}_sd")
    eng.tensor_scalar(out=seed[:], in0=t[:], scalar1=x_ap, scalar2=C0,
                      op0=OP.mult, op1=OP.add)
    sneg = pool.tile([128, 1], F32, tag=f"{tag}_s")
    eng.tensor_scalar(
        out=sneg[:], in0=x_ap, scalar1=-0.5, scalar2=None, op0=OP.mult
    )
    cur = seed
    for _ in range(iters):
        rr = pool.tile([128, 1], F32, tag=f"{tag}_rr")
        eng.tensor_tensor(out=rr[:], in0=cur[:], in1=cur[:], op=OP.mult)
        z = pool.tile([128, 1], F32, tag=f"{tag}_z")
        eng.tensor_scalar(
            out=z[:], in0=rr[:], scalar1=sneg[:, 0:1], scalar2=1.5,
            op0=OP.mult, op1=OP.add,
        )
        nxt = pool.tile([128, 1], F32, tag=f"{tag}_rn")
        eng.tensor_tensor(out=nxt[:], in0=cur[:], in1=z[:], op=OP.mult)
        cur = nxt
    return cur


def _wrap_pi(nc, pool, x_ap, width, tag, passes=3):
    """Branchless wrap of x into [-pi, pi]; each pass corrects one +-2pi."""
    PI = float(np.pi)
    TWO_PI = float(2 * np.pi)
    cur = x_ap
    for _ in range(passes):
        gt = pool.tile([128, width], F32, tag=f"{tag}_g")
        nc.vector.tensor_scalar(
            out=gt[:], in0=cur, scalar1=PI, scalar2=None, op0=OP.is_gt
        )
        t1 = pool.tile([128, width], F32, tag=f"{tag}_w")
        nc.vector.scalar_tensor_tensor(
            out=t1[:], in0=gt[:], scalar=-TWO_PI, in1=cur, op0=OP.mult, op1=OP.add
        )
        lt = pool.tile([128, width], F32, tag=f"{tag}_g")
        nc.vector.tensor_scalar(
            out=lt[:], in0=t1[:], scalar1=-PI, scalar2=None, op0=OP.is_lt
        )
        t2 = pool.tile([128, width], F32, tag=f"{tag}_w")
        nc.vector.scalar_tensor_tensor(
            out=t2[:], in0=lt[:], scalar=TWO_PI, in1=t1[:], op0=OP.mult, op1=OP.add
        )
        cur = t2[:]
    return cur


def _outer_bcast(ap, n):
    """AP reading t[p, j] broadcast over a new leading free dim of size n."""
    return bass.AP(tensor=ap.tensor, offset=ap.offset, ap=[ap.ap[0], [0, n], ap.ap[1]])


def build_nc():
    nc = bacc.Bacc("TRN2", target_bir_lowering=False, debug=False)

    # ---------------- DRAM tensors (per-core inputs) ----------------
    wi_d = nc.dram_tensor("wi", [NPOS, 1], I32, kind="ExternalInput")
    au_d = nc.dram_tensor("au", [NPOS, AUDIO], F32, kind="ExternalInput")
    lute_d = nc.dram_tensor("lute", [VOCAB, EMBD], F32, kind="ExternalInput")
    # lutp is [phase_tab0 | phase_tab1] concatenated host-side
    lutp_d = nc.dram_tensor("lutp", [VOCAB, D1 + D2], F32, kind="ExternalInput")
    pack0_d = nc.dram_tensor("pack0", [128, PACK0_COLS], F32, kind="ExternalInput")
    pack1_d = nc.dram_tensor("pack1", [128, 1536], BF16, kind="ExternalInput")
    pack2_d = nc.dram_tensor("pack2", [128, PACK2_COLS], F32, kind="ExternalInput")
    packb_d = nc.dram_tensor("packb", [128, PACKB_COLS], BF16, kind="ExternalInput")
    blin_d = nc.dram_tensor("blin", [1, D1], F32, kind="ExternalInput")
    mw_d = nc.dram_tensor("mw", [1, 2], F32, kind="ExternalInput")
    y_d = nc.dram_tensor("y", [BL, 1], F32, kind="ExternalOutput")

    with tile.TileContext(nc) as tc, ExitStack() as ctx:
        cpool = ctx.enter_context(tc.tile_pool(name="const", bufs=1))
        wpool = ctx.enter_context(tc.tile_pool(name="work", bufs=2))
        npool = ctx.enter_context(tc.tile_pool(name="newton", bufs=2))
        lpool = ctx.enter_context(tc.tile_pool(name="lstm", bufs=3))
        # PSUM budget is 8 banks; share one tag per pool so slots are reused.
        ps_g = ctx.enter_context(tc.tile_pool(name="psg", bufs=2, space="PSUM"))
        ps_a = ctx.enter_context(tc.tile_pool(name="psa", bufs=3, space="PSUM"))
        ps_b = ctx.enter_context(tc.tile_pool(name="psb", bufs=3, space="PSUM"))

        # ---------------- bulk loads (criticality order) ----------------
        # wi goes on the DVE queue so pack1 (which gates the GX matmuls) is
        # the first SP issue.
        # wi rides the sync queue FIRST: 512B must not queue behind the
        # ~700KB of pack DMAs on the DMA rings (it gates the gathers).
        wi = cpool.tile([NPOS, 1], I32, tag="wi")
        nc.sync.dma_start(wi[:], wi_d[:])
        pack1 = cpool.tile([128, 1536], BF16, tag="pack1")
        nc.sync.dma_start(pack1[:], pack1_d[:])
        packb = cpool.tile([128, PACKB_COLS], BF16, tag="packb")
        nc.sync.dma_start(packb[:], packb_d[:])
        pack0 = cpool.tile([128, PACK0_COLS], F32, tag="pack0")
        nc.sync.dma_start(pack0[:], pack0_d[:])
        pack2 = cpool.tile([128, PACK2_COLS], F32, tag="pack2")
        nc.sync.dma_start(pack2[:], pack2_d[:])

        def p0(name, nrow=128):
            c0, ncol = PACK0[name]
            return pack0[0:nrow, c0 : c0 + ncol]

        def p2(name, nrow=128):
            c0, ncol = PACK2[name]
            return pack2[0:nrow, c0 : c0 + ncol]

        def pb(name, nrow=128):
            c0, ncol = PACKB[name]
            return packb[0:nrow, c0 : c0 + ncol]

        wihta = pack1[:, 0:512]
        wihtb = pack1[:, 512:1024]
        wihtc = pack1[0:45, 1024:1536]
        whht = pb("whht")
        identb = pb("identb")
        wlinb = pb("wlinb")
        s1tb = pb("s1tb")
        s2tb = pb("s2tb")
        mr = p2("mr")
        mi = p2("mi")
        ident = p0("ident")
        fw1t = p0("fw1t")
        fw2t = p0("fw2t", 64)
        w1b = p0("w1b", 75)
        w2b = p0("w2b", 17)
        w3b = p0("w3b", 17)
        fb1 = p0("fb1", 64)
        fb2 = p0("fb2", 64)
        fw3t = p0("fw3t", 64)
        fb3 = p0("fb3", 1)
        au = cpool.tile([NPOS, AUDIO], F32, tag="au")
        nc.sync.dma_start(au[:], au_d[:])

        # partition-broadcast loads
        blin_bc = cpool.tile([128, D1], F32, tag="blin_bc")
        bap = blin_d[:]
        nc.sync.dma_start(
            blin_bc[:],
            bass.AP(tensor=bap.tensor, offset=bap.offset, ap=[[0, 128], bap.ap[1]]),
        )
        mwb = cpool.tile([128, 2], F32, tag="mwb")
        map_ = mw_d[:]
        nc.sync.dma_start(
            mwb[:],
            bass.AP(tensor=map_.tensor, offset=map_.offset, ap=[[0, 128], map_.ap[1]]),
        )

        # ---------------- gathers: emb first, then phases ------
        # emb gathers straight to bf16 via the casting SWDGE path
        emb = cpool.tile([NPOS, EMBD], BF16, tag="embg")
        nc.gpsimd.indirect_dma_start(
            out=emb[:], out_offset=None, in_=lute_d[:],
            in_offset=bass.IndirectOffsetOnAxis(ap=wi[:, 0:1], axis=0),
        )
        gathp = cpool.tile([NPOS, D1 + D2], F32, tag="gathp")
        nc.gpsimd.indirect_dma_start(
            out=gathp[:], out_offset=None, in_=lutp_d[:],
            in_offset=bass.IndirectOffsetOnAxis(ap=wi[:, 0:1], axis=0),
        )

        # ---------------- trig (ACT set #1) ----------------
        # HW Sin domain is [-pi, pi]: wrap both phase tabs together (3 passes
        # cover +-7pi), one extra shifted wrap gives the cos argument.
        PI = float(np.pi)
        TWO_PI = float(2 * np.pi)

        def wrap3(src_ap, width, tag):
            cur = src_ap
            for p in range(3):
                t = wpool.tile([128, width], F32, tag=f"{tag}_w")
                nc.vector.add_range_wrap(
                    out=t[:], in_=cur, shift=0.0, bound=PI, period=TWO_PI
                )
                cur = t[:]
            return cur

        phw = wrap3(gathp[:, 0 : D1 + D2], D1 + D2, "wr")
        phc = wpool.tile([128, D1 + D2], F32, tag="ca")
        nc.vector.add_range_wrap(
            out=phc[:], in_=phw, shift=HALF_PI, bound=PI, period=TWO_PI
        )
        sinall = cpool.tile([NPOS, D1 + D2], F32, tag="sinall")
        nc.scalar.activation(out=sinall[:], in_=phw, func=AF.Sin)
        cosall = cpool.tile([NPOS, D1 + D2], F32, tag="cosall")
        nc.scalar.activation(out=cosall[:], in_=phc[:], func=AF.Sin)
        sin0 = sinall[:, 0:D1]
        sin1 = sinall[:, D1 : D1 + D2]
        cos0 = cosall[:, 0:D1]
        cos1 = cosall[:, D1 : D1 + D2]

        # dummy exp-set trigger -- forced AFTER both Sin ops (ACT table
        # order), so the exp_and_others table load happens off the LSTM path.
        jj = wpool.tile([128, 1], F32, tag="jj")
        nc.vector.scalar_tensor_tensor(
            out=jj[:], in0=sinall[:, 0:1], scalar=0.0, in1=cosall[:, 0:1],
            op0=OP.mult, op1=OP.mult,
        )
        # ---- from here on, every ACT op uses exp_and_others (exp/tanh/square)
        dxp = wpool.tile([128, 1], F32, tag="dxp")
        nc.scalar.activation(out=dxp[:], in_=jj[:], func=AF.Exp)
        # modality softmax is host-precomputed into mwb
        mw0 = mwb[:, 0:1]
        mw1 = mwb[:, 1:2]

        # ---------------- embedding transpose + GX matmuls ----------------
        embT0 = cpool.tile([128, NPOS], BF16, tag="embT0")
        pst = ps_b.tile([128, 128], BF16, tag="psb")
        nc.tensor.transpose(pst[:], emb[:, 0:128], identb)
        nc.vector.tensor_copy(out=embT0[:], in_=pst[:])
        embT1 = cpool.tile([128, NPOS], BF16, tag="embT1")
        pst = ps_b.tile([128, 128], BF16, tag="psb")
        nc.tensor.transpose(pst[:], emb[:, 128:256], identb)
        nc.vector.tensor_copy(out=embT1[:], in_=pst[:])
        embT2 = cpool.tile([64, NPOS], BF16, tag="embT2")
        nc.vector.memset(embT2[:], 1.0)
        pst44 = ps_b.tile([44, 128], BF16, tag="psb")
        nc.tensor.transpose(pst44[:], emb[:, 256:300], identb)
        nc.vector.tensor_copy(out=embT2[0:44, :], in_=pst44[:])

        # GXT[hid, gate, pos] holds pre-activation z' (sigma-fold pre-scaled).
        # Two position blocks so early LSTM steps start before all GX is done.
        GXT = cpool.tile([HID, 4, NPOS], BF16, tag="GXT")
        for blk, (p0, p1) in enumerate(((0, 32), (32, NPOS))):
            for g in range(4):
                gsl = slice(HID * g, HID * (g + 1))
                pg = ps_a.tile([128, p1 - p0], F32, tag="psa")
                nc.tensor.matmul(
                    pg[:], wihta[:, gsl], embT0[:, p0:p1], start=True, stop=False
                )
                nc.tensor.matmul(
                    pg[:], wihtb[:, gsl], embT1[:, p0:p1], start=False, stop=False
                )
                nc.tensor.matmul(
                    pg[:], wihtc[:, gsl], embT2[0:45, p0:p1], start=False, stop=True
                )
                nc.vector.tensor_copy(out=GXT[:, g, p0:p1], in_=pg[:])

        # ---------------- audio MLP (feature-major) ----------------
        auT = cpool.tile([96, NPOS], F32, tag="auT")
        nc.vector.memset(auT[:], 1.0)
        psau = ps_b.tile([AUDIO, NPOS], F32, tag="psb")
        nc.tensor.transpose(psau[:], au[:], ident)
        nc.vector.tensor_copy(out=auT[0:AUDIO, :], in_=psau[:])

        a1b = cpool.tile([32, NPOS], F32, tag="a1b")
        nc.vector.memset(a1b[:], 1.0)
        psm = ps_b.tile([D2, NPOS], F32, tag="psb")
        nc.tensor.matmul(psm[:], w1b, auT[0 : AUDIO + 1, :], start=True, stop=True)
        nc.vector.tensor_scalar(
            out=a1b[0:D2, :], in0=psm[:], scalar1=0.0, scalar2=None, op0=OP.max
        )
        a2b = cpool.tile([32, NPOS], F32, tag="a2b")
        nc.vector.memset(a2b[:], 1.0)
        psm = ps_b.tile([D2, NPOS], F32, tag="psb")
        nc.tensor.matmul(psm[:], w2b, a1b[0 : D2 + 1, :], start=True, stop=True)
        nc.vector.tensor_scalar(
            out=a2b[0:D2, :], in0=psm[:], scalar1=0.0, scalar2=None, op0=OP.max
        )
        amp1T = cpool.tile([D2, NPOS], F32, tag="amp1T")
        psm = ps_b.tile([D2, NPOS], F32, tag="psb")
        nc.tensor.matmul(psm[:], w3b, a2b[0 : D2 + 1, :], start=True, stop=True)
        nc.vector.tensor_scalar(
            out=amp1T[:], in0=psm[:], scalar1=0.0, scalar2=None, op0=OP.max
        )
        # transpose to position-major
        amp1 = cpool.tile([NPOS, D2], F32, tag="amp1")
        ps16 = ps_b.tile([128, D2], F32, tag="psb")
        nc.tensor.transpose(ps16[:], amp1T[:], ident[0:D2, 0:D2])
        nc.vector.tensor_copy(out=amp1[:], in_=ps16[:])

        # n1 (norm) via prescaled Newton; the normalization itself is factored
        junk16 = wpool.tile([128, D2], F32, tag="junk16")
        s1x = wpool.tile([128, 1], F32, tag="s1x")
        nc.vector.scalar_tensor_tensor(
            out=junk16[:], in0=amp1[:], scalar=64.0, in1=amp1[:],
            op0=OP.mult, op1=OP.mult, accum_out=s1x[:, 0:1],
        )
        r1n = _newton_rsqrt(nc, npool, s1x[:], iters=3, tag="n1")
        n1 = wpool.tile([128, 1], F32, tag="n1v")
        nc.vector.tensor_scalar(
            out=n1[:], in0=s1x[:], scalar1=r1n[:, 0:1], scalar2=0.125,
            op0=OP.mult, op1=OP.mult,
        )
        wpart = cpool.tile([128, 1], F32, tag="wpart")
        nc.vector.tensor_scalar(
            out=wpart[:], in0=n1[:], scalar1=mw1[:, 0:1], scalar2=None, op0=OP.mult
        )
        # UNnormalized branch-1 vectors (1/(n1+eps) factored out, applied at M)
        r1v = cpool.tile([NPOS, D2], F32, tag="r1v")
        nc.vector.tensor_tensor(out=r1v[:], in0=amp1[:], in1=cos1[:], op=OP.mult)
        i1v = cpool.tile([NPOS, D2], F32, tag="i1v")
        nc.vector.tensor_tensor(out=i1v[:], in0=amp1[:], in1=sin1[:], op=OP.mult)
        uv = cpool.tile([NPOS, D2], F32, tag="uv")
        nc.vector.tensor_tensor(out=uv[:], in0=r1v[:], in1=i1v[:], op=OP.subtract)
        vv = cpool.tile([NPOS, D2], F32, tag="vv")
        nc.vector.tensor_tensor(out=vv[:], in0=r1v[:], in1=i1v[:], op=OP.add)

        # Force LSTM step-0 (first exp-set ACT op) after all four Sin outputs
        # so the trig table set is loaded exactly once, before exp_and_others.
        tg_a = wpool.tile([128, BL], F32, tag="tg_a")
        nc.vector.scalar_tensor_tensor(
            out=tg_a[:], in0=cos0[:, 0:BL], scalar=0.0, in1=sin0[:, 0:BL],
            op0=OP.mult, op1=OP.mult,
        )
        tg_b = wpool.tile([128, BL], F32, tag="tg_b")
        nc.vector.scalar_tensor_tensor(
            out=tg_b[:], in0=cos1[:, 0:BL], scalar=0.0, in1=sin1[:, 0:BL],
            op0=OP.mult, op1=OP.mult,
        )
        # scalar=dxp (==1) anchors the dummy-exp table load before step 0
        tg_c = wpool.tile([128, BL], BF16, tag="tg_c")
        nc.vector.scalar_tensor_tensor(
            out=tg_c[:], in0=tg_a[:], scalar=dxp[:, 0:1], in1=tg_b[:],
            op0=OP.mult, op1=OP.add,
        )
        nc.vector.scalar_tensor_tensor(
            out=GXT[:, 0, 0:BL], in0=tg_c[:], scalar=1.0, in1=GXT[:, 0, 0:BL],
            op0=OP.mult, op1=OP.add,
        )

        # ---------------- LSTM (hidden-major, doubled states) ----------------
        # bf16 h-state and recurrent weights: single-pass PE matmuls w/ FWL.
        H2 = cpool.tile([HID, NPOS], BF16, tag="H2")
        c2_prev = None
        for s in range(S):
            gx3 = GXT[:, :, s * BL : (s + 1) * BL]  # (128, 4, 4)
            gates = lpool.tile([128, 4, BL], F32, tag="gates")
            if s == 0:
                nc.scalar.activation(out=gates[:], in_=gx3, func=AF.Tanh)
            else:
                gp = ps_g.tile([128, 4, BL], F32, tag="gp")
                nc.tensor.matmul(gp[:], identb, gx3, start=True, stop=False)
                hprev = H2[:, (s - 1) * BL : s * BL]
                for g in range(4):
                    nc.tensor.matmul(
                        gp[:, g, :], whht[:, HID * g : HID * (g + 1)], hprev,
                        start=False, stop=(g == 3),
                    )
                nc.scalar.activation(out=gates[:], in_=gp[:], func=AF.Tanh)
            ti, tf = gates[:, 0, :], gates[:, 1, :]
            to, tg = gates[:, 2, :], gates[:, 3, :]
            q2 = lpool.tile([128, BL], F32, tag="q2")
            nc.vector.scalar_tensor_tensor(
                out=q2[:], in0=ti, scalar=1.0, in1=tg, op0=OP.add, op1=OP.mult
            )
            if s == 0:
                c2 = q2
            else:
                q1 = lpool.tile([128, BL], F32, tag="q1")
                nc.vector.scalar_tensor_tensor(
                    out=q1[:], in0=tf, scalar=1.0, in1=c2_prev[:],
                    op0=OP.add, op1=OP.mult,
                )
                c2 = lpool.tile([128, BL], F32, tag="c2")
                nc.vector.scalar_tensor_tensor(
                    out=c2[:], in0=q1[:], scalar=0.5, in1=q2[:],
                    op0=OP.mult, op1=OP.add,
                )
            th = lpool.tile([128, BL], F32, tag="th")
            nc.scalar.activation(out=th[:], in_=c2[:], func=AF.Tanh, scale=0.5)
            nc.vector.scalar_tensor_tensor(
                out=H2[:, s * BL : (s + 1) * BL], in0=to, scalar=1.0, in1=th[:],
                op0=OP.add, op1=OP.mult,
            )
            c2_prev = c2

        # ------------- measurement kernel norm (factored, applied at featT) ----
        junk256 = wpool.tile([128, DIM], F32, tag="junk256")
        skr = wpool.tile([128, 1], F32, tag="skr")
        nc.vector.scalar_tensor_tensor(
            out=junk256[:], in0=mr, scalar=1.0 / 128.0, in1=mr,
            op0=OP.mult, op1=OP.mult, accum_out=skr[:, 0:1],
        )
        ski = wpool.tile([128, 1], F32, tag="ski")
        nc.vector.scalar_tensor_tensor(
            out=junk256[:], in0=mi, scalar=1.0 / 128.0, in1=mi,
            op0=OP.mult, op1=OP.mult, accum_out=ski[:, 0:1],
        )
        kx = wpool.tile([128, 1], F32, tag="kx")
        nc.vector.tensor_tensor(out=kx[:], in0=skr[:], in1=ski[:], op=OP.add)
        # rkn ~ rsqrt(kn2/128); true rsqrt(kn2)^2 = rkn^2/128
        rkn = _newton_rsqrt(nc, npool, kx[:], iters=4, tag="kn")
        rkn2 = cpool.tile([128, 1], F32, tag="rkn2")
        nc.vector.tensor_scalar(
            out=rkn2[:], in0=rkn[:], scalar1=rkn[:, 0:1], scalar2=1.0 / 128.0,
            op0=OP.mult, op1=OP.mult,
        )
        # measurement tiles arranged so (P+T) and (R-Q) accumulate in PSUM:
        # vrviA_k = [vrT_k | -viT_k] (moving for real chunks),
        # vrviB_k = [viT_k |  vrT_k] (moving for imag chunks)
        vrviA = []
        vrviB = []
        for k in range(2):
            ta = cpool.tile([128, 2 * UNITS], BF16, tag=f"vrviA{k}")
            tb = cpool.tile([128, 2 * UNITS], BF16, tag=f"vrviB{k}")
            pvr = ps_b.tile([128, 128], F32, tag="psb")
            nc.tensor.transpose(pvr[:], mr[:, 128 * k : 128 * (k + 1)], ident)
            pvi = ps_b.tile([128, 128], F32, tag="psb")
            nc.tensor.transpose(pvi[:], mi[:, 128 * k : 128 * (k + 1)], ident)
            nc.scalar.copy(out=ta[:, 0:128], in_=pvr[:])
            nc.vector.tensor_scalar(
                out=ta[:, 128:256], in0=pvi[:], scalar1=-1.0, scalar2=None,
                op0=OP.mult,
            )
            nc.scalar.copy(out=tb[:, 0:128], in_=pvi[:])
            nc.vector.tensor_copy(out=tb[:, 128:256], in_=pvr[:])
            vrviA.append(ta)
            vrviB.append(tb)

        # ---------------- amp0 / weight path ----------------
        amp0p = ps_b.tile([NPOS, D1], F32, tag="psb")
        nc.tensor.matmul(amp0p[:], H2[:], wlinb, start=True, stop=True)
        amp0 = cpool.tile([NPOS, D1], F32, tag="amp0")
        nc.vector.tensor_tensor(out=amp0[:], in0=amp0p[:], in1=blin_bc[:], op=OP.add)
        junk16b = wpool.tile([128, D1], F32, tag="junk16b")
        s0x = wpool.tile([128, 1], F32, tag="s0x")
        nc.vector.scalar_tensor_tensor(
            out=junk16b[:], in0=amp0[:], scalar=8.0, in1=amp0[:],
            op0=OP.mult, op1=OP.mult, accum_out=s0x[:, 0:1],
        )
        # sqrt-free measurement scale: csq = 1/(s0*s1) = 512/(s0x*s1x)
        csp = wpool.tile([128, 1], F32, tag="csp")
        nc.vector.tensor_scalar(
            out=csp[:], in0=s0x[:], scalar1=s1x[:, 0:1], scalar2=1.0 / 512.0,
            op0=OP.mult, op1=OP.mult,
        )
        csq = cpool.tile([128, 1], F32, tag="csq")
        nc.vector.reciprocal(out=csq[:], in_=csp[:])
        # ---------------- tensor product -> realim (pos-major) ----------------
        # UNnormalized: the 1/(n0+eps), 1/(n1+eps) factors are applied at M.
        r0v = wpool.tile([NPOS, D1], F32, tag="r0v")
        nc.vector.tensor_tensor(out=r0v[:], in0=amp0[:], in1=cos0[:], op=OP.mult)
        i0v = wpool.tile([NPOS, D1], F32, tag="i0v")
        nc.vector.tensor_tensor(out=i0v[:], in0=amp0[:], in1=sin0[:], op=OP.mult)

        realim = cpool.tile([NPOS, 2 * DIM], BF16, tag="realim")
        tmpA = wpool.tile([NPOS, DIM], F32, tag="tmpA")
        tmpB = wpool.tile([NPOS, DIM], F32, tag="tmpB")
        # real = r0 (x) u - i0 (x) v   ; imag = r0 (x) v + i0 (x) u
        nc.vector.tensor_tensor(
            out=tmpA[:].rearrange("p (i j) -> p i j", j=D2),
            in0=r0v[:].to_broadcast([NPOS, D1, D2]),
            in1=_outer_bcast(uv[:], D1), op=OP.mult,
        )
        nc.vector.tensor_tensor(
            out=tmpB[:].rearrange("p (i j) -> p i j", j=D2),
            in0=i0v[:].to_broadcast([NPOS, D1, D2]),
            in1=_outer_bcast(vv[:], D1), op=OP.mult,
        )
        nc.vector.tensor_tensor(
            out=realim[:, 0:DIM], in0=tmpA[:], in1=tmpB[:], op=OP.subtract
        )
        tmpC = wpool.tile([NPOS, DIM], F32, tag="tmpC")
        tmpD = wpool.tile([NPOS, DIM], F32, tag="tmpD")
        nc.vector.tensor_tensor(
            out=tmpC[:].rearrange("p (i j) -> p i j", j=D2),
            in0=r0v[:].to_broadcast([NPOS, D1, D2]),
            in1=_outer_bcast(vv[:], D1), op=OP.mult,
        )
        nc.vector.tensor_tensor(
            out=tmpD[:].rearrange("p (i j) -> p i j", j=D2),
            in0=i0v[:].to_broadcast([NPOS, D1, D2]),
            in1=_outer_bcast(uv[:], D1), op=OP.mult,
        )
        nc.vector.tensor_tensor(
            out=realim[:, DIM : 2 * DIM], in0=tmpC[:], in1=tmpD[:], op=OP.add
        )

        # transpose realim -> rimT (dim-major), 4 chunks of 128
        rimT = cpool.tile([128, 4 * 128], BF16, tag="rimT")
        for q in range(4):
            pst = ps_b.tile([128, 128], BF16, tag="psb")
            nc.tensor.transpose(pst[:], realim[:, 128 * q : 128 * (q + 1)], identb)
            nc.scalar.copy(out=rimT[:, 128 * q : 128 * (q + 1)], in_=pst[:])

        # [P+T | R-Q] accumulates across all four matmuls in one PSUM tile
        ps_u = ps_a.tile([NPOS, 2 * UNITS], F32, tag="psa")
        nc.tensor.matmul(
            ps_u[:], rimT[:, 0:128], vrviA[0][:], start=True, stop=False
        )
        nc.tensor.matmul(
            ps_u[:], rimT[:, 128:256], vrviA[1][:], start=False, stop=False
        )
        nc.tensor.matmul(
            ps_u[:], rimT[:, 256:384], vrviB[0][:], start=False, stop=False
        )
        nc.tensor.matmul(
            ps_u[:], rimT[:, 384:512], vrviB[1][:], start=False, stop=True
        )
        # weight-path newton on DVE, emitted after the real-half: runs while
        # the PE does the rimT transposes + PQRT matmuls
        r0n = _newton_rsqrt_q(nc, npool, s0x[:], iters=3, tag="n0")
        n0 = wpool.tile([128, 1], F32, tag="n0v")
        nc.vector.tensor_scalar(
            out=n0[:], in0=s0x[:], scalar1=r0n[:, 0:1], scalar2=INV_SQRT8,
            op0=OP.mult, op1=OP.mult,
        )
        weight = wpool.tile([128, 1], F32, tag="weight")
        nc.vector.scalar_tensor_tensor(
            out=weight[:], in0=n0[:], scalar=mw0[:, 0:1], in1=wpart[:],
            op0=OP.mult, op1=OP.add,
        )
        wb16 = wpool.tile([128, 1], BF16, tag="wb16")
        nc.vector.tensor_copy(out=wb16[:], in_=weight[:])
        sq1 = wpool.tile([NPOS, UNITS], F32, tag="sq1")
        nc.scalar.activation(out=sq1[:], in_=ps_u[:, 0:UNITS], func=AF.Square)
        sq2 = wpool.tile([NPOS, UNITS], F32, tag="sq2")
        nc.scalar.activation(
            out=sq2[:], in_=ps_u[:, UNITS : 2 * UNITS], func=AF.Square
        )
        msu = wpool.tile([NPOS, UNITS], F32, tag="msu")
        nc.vector.tensor_tensor(out=msu[:], in0=sq1[:], in1=sq2[:], op=OP.add)
        msr = cpool.tile([NPOS, UNITS], BF16, tag="msr")
        nc.vector.tensor_scalar(
            out=msr[:], in0=msu[:], scalar1=csq[:, 0:1], scalar2=None,
            op0=OP.mult,
        )

        # windowed weights [sh4(w), sh8(w), w] assembled in PSUM, then softmax
        vp = ps_b.tile([128, 3], F32, tag="psb")
        nc.tensor.matmul(vp[:, 0:1], s1tb, wb16[:], start=True, stop=True)
        nc.tensor.matmul(vp[:, 1:2], s2tb, wb16[:], start=True, stop=True)
        nc.tensor.matmul(vp[:, 2:3], identb, wb16[:], start=True, stop=True)
        e3 = wpool.tile([128, 3], F32, tag="e3")
        esum = wpool.tile([128, 1], F32, tag="esum")
        nc.scalar.activation(
            out=e3[:], in_=vp[:], func=AF.Exp, accum_out=esum[:, 0:1]
        )
        res = wpool.tile([128, 1], F32, tag="res")
        nc.vector.reciprocal(out=res[:], in_=esum[:])
        ww = wpool.tile([128, 3], F32, tag="ww")
        nc.vector.tensor_scalar(
            out=ww[:], in0=e3[:], scalar1=res[:, 0:1], scalar2=None, op0=OP.mult
        )

        # n-gram mixing: m3 = ww0*M + ww1*shift4(M) + ww2*shift8(M)
        ps_ms = ps_a.tile([NPOS, 2 * UNITS], F32, tag="psa")
        nc.tensor.matmul(ps_ms[:, 0:UNITS], s1tb, msr[:], start=True, stop=True)
        nc.tensor.matmul(
            ps_ms[:, UNITS : 2 * UNITS], s2tb, msr[:], start=True, stop=True
        )
        t1m = wpool.tile([NPOS, UNITS], F32, tag="t1m")
        nc.vector.tensor_scalar(
            out=t1m[:], in0=ps_ms[:, 0:UNITS], scalar1=ww[:, 0:1], scalar2=None,
            op0=OP.mult,
        )
        m3a = wpool.tile([NPOS, UNITS], F32, tag="m3a")
        nc.vector.scalar_tensor_tensor(
            out=m3a[:], in0=ps_ms[:, UNITS : 2 * UNITS], scalar=ww[:, 1:2], in1=t1m[:],
            op0=OP.mult, op1=OP.add,
        )
        m3 = wpool.tile([NPOS, UNITS], F32, tag="m3")
        nc.vector.scalar_tensor_tensor(
            out=m3[:], in0=msr[:], scalar=ww[:, 2:3], in1=m3a[:],
            op0=OP.mult, op1=OP.add,
        )
        mmx = wpool.tile([NPOS, UNITS], BF16, tag="mmx")
        nc.vector.tensor_tensor(out=mmx[:], in0=msr[:], in1=m3[:], op=OP.max)

        # max over positions per batch: transpose then reduce over s
        ps_mt = ps_b.tile([UNITS, NPOS], BF16, tag="psb")
        nc.tensor.transpose(ps_mt[:], mmx[:], identb)
        featU = wpool.tile([UNITS, BL], F32, tag="featU")
        nc.vector.tensor_reduce(
            out=featU[:], in_=ps_mt[:].rearrange("p (s b) -> p b s", b=BL),
            axis=mybir.AxisListType.X, op=OP.max,
        )
        featT = wpool.tile([UNITS, BL], F32, tag="featT")
        nc.vector.tensor_scalar(
            out=featT[:], in0=featU[:], scalar1=rkn2[:, 0:1], scalar2=None,
            op0=OP.mult,
        )

        # ---------------- final MLP ----------------
        y1p = ps_b.tile([CELL, BL], F32, tag="psb")
        nc.tensor.matmul(y1p[:], fw1t, featT[:], start=True, stop=True)
        y1 = wpool.tile([CELL, BL], F32, tag="y1")
        nc.vector.tensor_scalar(
            out=y1[:], in0=y1p[:], scalar1=fb1[:, 0:1], scalar2=0.0,
            op0=OP.add, op1=OP.max,
        )
        y2p = ps_b.tile([CELL, BL], F32, tag="psb")
        nc.tensor.matmul(y2p[:], fw2t, y1[:], start=True, stop=True)
        y2 = wpool.tile([CELL, BL], F32, tag="y2")
        nc.vector.tensor_scalar(
            out=y2[:], in0=y2p[:], scalar1=fb2[:, 0:1], scalar2=0.0,
            op0=OP.add, op1=OP.max,
        )
        y3p = ps_b.tile([1, BL], F32, tag="psb")
        nc.tensor.matmul(y3p[:], fw3t, y2[:], start=True, stop=True)
        ysb = wpool.tile([1, BL], F32, tag="ysb")
        nc.vector.tensor_scalar(
            out=ysb[:], in0=y3p[:], scalar1=fb3[0:1, 0:1], scalar2=None, op0=OP.add
        )
        nc.sync.dma_start(y_d[:], ysb[:])

    nc.compile()
    return nc


_NC = None


def _get_nc():
    global _NC
    if _NC is None:
        _NC = build_nc()
    return _NC


def _padrows(a, rows):
    out = np.zeros((rows, a.shape[1]), np.float32)
    out[: a.shape[0]] = a
    return out


def make_in_maps(inputs):
    """Host-side layout prep (value-preserving transforms only)."""
    import ml_dtypes

    f32 = np.float32
    bf16 = ml_dtypes.bfloat16
    wi_full = np.asarray(inputs["word_indexes"]).astype(np.int32)  # (B, S)
    au_full = np.asarray(inputs["audio"], dtype=f32)  # (B, S, A)
    lute = np.ascontiguousarray(np.asarray(inputs["lookup_table"], dtype=f32))
    lutp = np.ascontiguousarray(
        np.concatenate(
            [
                np.asarray(inputs["phase_tab0"], dtype=f32),
                np.asarray(inputs["phase_tab1"], dtype=f32),
            ],
            axis=1,
        )
    )
    w_ih = np.asarray(inputs["w_ih"], dtype=f32)
    w_hh = np.asarray(inputs["w_hh"], dtype=f32)
    b_ih = np.asarray(inputs["b_ih"], dtype=f32)
    b_hh = np.asarray(inputs["b_hh"], dtype=f32)
    w_lin = np.asarray(inputs["w_lin"], dtype=f32)
    b_lin = np.asarray(inputs["b_lin"], dtype=f32)

    # gate reorder [i, f, o, g] + sigma->tanh fold (x0.5 on i,f,o rows);
    # extra x0.5 on all w_hh entries for the doubled hidden state H2=2h.
    perm = np.concatenate(
        [np.arange(0, 128), np.arange(128, 256), np.arange(384, 512),
         np.arange(256, 384)]
    )
    gsc = np.concatenate([np.full(384, 0.5, f32), np.ones(128, f32)])
    w_ih_p = w_ih[perm] * gsc[:, None]
    b_p = (b_ih + b_hh)[perm] * gsc
    wihT = np.concatenate([w_ih_p.T, b_p[None, :]], axis=0).astype(f32)  # (301,512)
    whht = (w_hh[perm] * gsc[:, None] * 0.5).T.astype(f32)  # (128, 512)

    pack1 = np.concatenate(
        [wihT[0:128], wihT[128:256], _padrows(wihT[256:301], 128)], axis=1
    )
    pack1 = np.ascontiguousarray(pack1.astype(bf16))

    w1 = np.asarray(inputs["w1"], dtype=f32)
    w2 = np.asarray(inputs["w2"], dtype=f32)
    w3 = np.asarray(inputs["w3"], dtype=f32)
    b1 = np.asarray(inputs["b1"], dtype=f32)
    b2 = np.asarray(inputs["b2"], dtype=f32)
    b3 = np.asarray(inputs["b3"], dtype=f32)
    parts0 = {
        "ident": np.eye(128, dtype=f32),
        "fw1t": np.asarray(inputs["fw1"], dtype=f32).T,
        "fw2t": np.asarray(inputs["fw2"], dtype=f32).T,
        "w1b": np.concatenate([w1.T, b1[None, :]], 0),
        "w2b": np.concatenate([w2.T, b2[None, :]], 0),
        "w3b": np.concatenate([w3.T, b3[None, :]], 0),
        "fb1": np.asarray(inputs["fb1"], dtype=f32).reshape(CELL, 1),
        "fb2": np.asarray(inputs["fb2"], dtype=f32).reshape(CELL, 1),
        "fw3t": np.asarray(inputs["fw3"], dtype=f32).T.reshape(CELL, 1),
        "fb3": np.asarray(inputs["fb3"], dtype=f32).reshape(1, 1),
    }
    pack0 = np.zeros((128, PACK0_COLS), f32)
    for name, (c0, ncol) in PACK0.items():
        arr = np.asarray(parts0[name], dtype=f32)
        assert arr.shape[1] == ncol and arr.shape[0] <= 128, (name, arr.shape)
        pack0[: arr.shape[0], c0 : c0 + ncol] = arr
    pack0 = np.ascontiguousarray(pack0)
    parts2 = {
        "mr": np.asarray(inputs["meas_r"], dtype=f32),
        "mi": np.asarray(inputs["meas_i"], dtype=f32),
    }
    pack2 = np.zeros((128, PACK2_COLS), f32)
    for name, (c0, ncol) in PACK2.items():
        arr = np.asarray(parts2[name], dtype=f32)
        pack2[: arr.shape[0], c0 : c0 + ncol] = arr
    pack2 = np.ascontiguousarray(pack2)

    partsb = {
        "identb": np.eye(128, dtype=f32),
        "whht": whht,
        "wlinb": (0.5 * w_lin).T,
        "s1tb": np.eye(128, k=4, dtype=f32).T,
        "s2tb": np.eye(128, k=8, dtype=f32).T,
    }
    packb = np.zeros((128, PACKB_COLS), f32)
    for name, (c0, ncol) in PACKB.items():
        arr = np.asarray(partsb[name], dtype=f32)
        packb[: arr.shape[0], c0 : c0 + ncol] = arr
    packb = np.ascontiguousarray(packb.astype(bf16))

    blin = np.ascontiguousarray(b_lin.reshape(1, D1), dtype=f32)
    mwraw = np.asarray(inputs["modality_weights"], dtype=np.float64).reshape(2)
    mwe = np.exp(mwraw - mwraw.max())
    mw = np.ascontiguousarray((mwe / mwe.sum()).reshape(1, 2).astype(f32))

    shared = dict(
        lute=lute, lutp=lutp, pack0=pack0, pack1=pack1, pack2=pack2,
        packb=packb, blin=blin, mw=mw,
    )
    in_maps = []
    for c in range(NCORES):
        bs = slice(BL * c, BL * (c + 1))
        # s-major position order: pos = s*BL + b
        wi_c = np.ascontiguousarray(wi_full[bs].T.reshape(NPOS, 1))
        au_c = np.ascontiguousarray(
            au_full[bs].transpose(1, 0, 2).reshape(NPOS, AUDIO)
        )
        m = dict(shared)
        m["wi"] = wi_c
        m["au"] = au_c
        in_maps.append(m)
    return in_maps


def kernel(**inputs):
    from concourse.bass_utils import run_bass_kernel_spmd

    nc = _get_nc()
    in_maps = make_in_maps(inputs)
    res = run_bass_kernel_spmd(nc, in_maps, core_ids=list(range(NCORES)))
    out = np.concatenate(
        [np.asarray(res.results[c]["y"]).reshape(BL, 1) for c in range(NCORES)], axis=0
    ).astype(np.float32)
    return out



# revision 45
# speedup vs baseline: 1.1668x; 1.1668x over previous
"""Trainium2 Bass kernel for nn_LocalMixtureNN (self-contained).

Strategy
--------
Pure data parallel over batch: 8 cores x 4 batches. Within a core the 128
(s, b) positions live on the 128 SBUF partitions (pos = s*4 + b, s-major so
n-gram window shifts are partition shifts by 4*k, realized as matmuls with
constant shift matrices).

The LSTM recurrence runs hidden-major: h/c state is (128 hid, 4 batch).
Sigmoid is folded into tanh (sigma(z) = (tanh(z/2)+1)/2) with all the 0.5
factors pre-folded into host-prepared weights, and doubled cell/hidden state
(C2 = 2c, H2 = 2h), so every activation in the whole kernel comes from the
single "exp_and_others" ACT table set (tanh + exp + square) plus one early
"trig" set load for sin/cos. All sqrt/rsqrt are division-free Newton
iterations on prescaled inputs (no sqrt table load), and every normalization
(1/(n0+eps), 1/(n1+eps), measurement-kernel 1/kn) is factored out of the
dependency spine and applied late as per-partition scales (on M per position,
on featT per unit), so the Newton chains hide behind the measurement matmuls.

The density-matrix measurement collapses algebraically: m[p,u] =
sum_k ww_k |v_u^H x_{p+k}|^2 with |v^H x|^2 = (P+T)^2 + (R-Q)^2 where
P,Q,R,T are four real matmuls (real/imag vectors x meas_r/meas_i^T).

Weights are shipped in two packed (128, X) DMAs to avoid per-tensor DMA
issue serialization.
"""

import numpy as np

try:
    import concourse  # noqa: F401
except ImportError:  # pragma: no cover
    import sys

    sys.path.insert(0, "/opt/trn_rl_repo")

from contextlib import ExitStack

import concourse.bass as bass
import concourse.bacc as bacc
import concourse.tile as tile
import concourse.mybir as mybir

dt = mybir.dt
F32 = dt.float32
R32 = dt.float32r
BF16 = dt.bfloat16
I32 = dt.int32
AF = mybir.ActivationFunctionType
OP = mybir.AluOpType

NCORES = 8
B, S = 32, 32
BL = B // NCORES  # 4 batches per core
NPOS = S * BL  # 128 positions per core, pos = s*BL + b
VOCAB, EMBD, AUDIO, HID = 5000, 300, 74, 128
D1 = 16
D2 = 16
DIM = D1 * D2  # 256
UNITS, CELL = 128, 64
HALF_PI = float(np.pi / 2)
SQRT8 = float(np.sqrt(8.0))
INV_SQRT8 = float(1.0 / np.sqrt(8.0))

# pack0: hot small fp32 constants (identity/MLP weights).
PACK0 = {}
_c = 0
for _name, _ncol in (
    ("ident", 128), ("fw1t", 64), ("fw2t", 64),
    ("w1b", 16), ("w2b", 16), ("w3b", 16),
    ("fb1", 1), ("fb2", 1), ("fw3t", 1), ("fb3", 1),
):
    PACK0[_name] = (_c, _ncol)
    _c += _ncol
PACK0_COLS = _c  # 308
# pack2: measurement kernel (fp32, needed late)
PACK2 = {}
_c = 0
for _name, _ncol in (("mr", 256), ("mi", 256)):
    PACK2[_name] = (_c, _ncol)
    _c += _ncol
PACK2_COLS = _c  # 512
# packB: bf16 weights (identity for bf16 matmuls, LSTM recurrent, amp0 proj,
# n-gram shift matrices)
PACKB = {}
_c = 0
for _name, _ncol in (
    ("identb", 128), ("whht", 512), ("wlinb", 16), ("s1tb", 128), ("s2tb", 128),
):
    PACKB[_name] = (_c, _ncol)
    _c += _ncol
PACKB_COLS = _c  # 912


def _newton_rsqrt(nc, pool, x_ap, iters, tag, eng=None):
    """r ~= rsqrt(x) for x (128,1), seed 2/(1+x), division-free Newton.

    eng selects the elementwise engine (nc.vector or nc.gpsimd); the seed
    reciprocal is DVE-only either way. Pool-engine safe (tensor_scalar /
    tensor_tensor only).
    """
    if eng is None:
        eng = nc.vector
    t = pool.tile([128, 1], F32, tag=f"{tag}_t")
    eng.tensor_scalar(out=t[:], in0=x_ap, scalar1=1.0, scalar2=None, op0=OP.add)
    r = pool.tile([128, 1], F32, tag=f"{tag}_r0")
    nc.vector.reciprocal(out=r[:], in_=t[:])
    r2 = pool.tile([128, 1], F32, tag=f"{tag}_r2")
    eng.tensor_scalar(out=r2[:], in0=r[:], scalar1=2.0, scalar2=None, op0=OP.mult)
    sneg = pool.tile([128, 1], F32, tag=f"{tag}_s")
    eng.tensor_scalar(
        out=sneg[:], in0=x_ap, scalar1=-0.5, scalar2=None, op0=OP.mult
    )
    cur = r2
    for _ in range(iters):
        rr = pool.tile([128, 1], F32, tag=f"{tag}_rr")
        eng.tensor_tensor(out=rr[:], in0=cur[:], in1=cur[:], op=OP.mult)
        z = pool.tile([128, 1], F32, tag=f"{tag}_z")
        eng.tensor_scalar(
            out=z[:], in0=rr[:], scalar1=sneg[:, 0:1], scalar2=1.5,
            op0=OP.mult, op1=OP.add,
        )
        nxt = pool.tile([128, 1], F32, tag=f"{tag}_rn")
        eng.tensor_tensor(out=nxt[:], in0=cur[:], in1=z[:], op=OP.mult)
        cur = nxt
    return cur


def _newton_rsqrt_q(nc, pool, x_ap, iters, tag, eng=None):
    """rsqrt via newton with quadratic minimax seed on x in [0.15, 2.6]."""
    if eng is None:
        eng = nc.vector
    C2, C1, C0 = 0.4290883, -1.76366757, 2.40807279
    t = pool.tile([128, 1], F32, tag=f"{tag}_t")
    eng.tensor_scalar(out=t[:], in0=x_ap, scalar1=C2, scalar2=C1,
                      op0=OP.mult, op1=OP.add)
    seed = pool.tile([128, 1], F32, tag=f"{tag}_sd")
    eng.tensor_scalar(out=seed[:], in0=t[:], scalar1=x_ap, scalar2=C0,
                      op0=OP.mult, op1=OP.add)
    sneg = pool.tile([128, 1], F32, tag=f"{tag}_s")
    eng.tensor_scalar(
        out=sneg[:], in0=x_ap, scalar1=-0.5, scalar2=None, op0=OP.mult
    )
    cur = seed
    for _ in range(iters):
        rr = pool.tile([128, 1], F32, tag=f"{tag}_rr")
        eng.tensor_tensor(out=rr[:], in0=cur[:], in1=cur[:], op=OP.mult)
        z = pool.tile([128, 1], F32, tag=f"{tag}_z")
        eng.tensor_scalar(
            out=z[:], in0=rr[:], scalar1=sneg[:, 0:1], scalar2=1.5,
            op0=OP.mult, op1=OP.add,
        )
        nxt = pool.tile([128, 1], F32, tag=f"{tag}_rn")
        eng.tensor_tensor(out=nxt[:], in0=cur[:], in1=z[:], op=OP.mult)
        cur = nxt
    return cur


def _wrap_pi(nc, pool, x_ap, width, tag, passes=3):
    """Branchless wrap of x into [-pi, pi]; each pass corrects one +-2pi."""
    PI = float(np.pi)
    TWO_PI = float(2 * np.pi)
    cur = x_ap
    for _ in range(passes):
        gt = pool.tile([128, width], F32, tag=f"{tag}_g")
        nc.vector.tensor_scalar(
            out=gt[:], in0=cur, scalar1=PI, scalar2=None, op0=OP.is_gt
        )
        t1 = pool.tile([128, width], F32, tag=f"{tag}_w")
        nc.vector.scalar_tensor_tensor(
            out=t1[:], in0=gt[:], scalar=-TWO_PI, in1=cur, op0=OP.mult, op1=OP.add
        )
        lt = pool.tile([128, width], F32, tag=f"{tag}_g")
        nc.vector.tensor_scalar(
            out=lt[:], in0=t1[:], scalar1=-PI, scalar2=None, op0=OP.is_lt
        )
        t2 = pool.tile([128, width], F32, tag=f"{tag}_w")
        nc.vector.scalar_tensor_tensor(
            out=t2[:], in0=lt[:], scalar=TWO_PI, in1=t1[:], op0=OP.mult, op1=OP.add
        )
        cur = t2[:]
    return cur


def _outer_bcast(ap, n):
    """AP reading t[p, j] broadcast over a new leading free dim of size n."""
    return bass.AP(tensor=ap.tensor, offset=ap.offset, ap=[ap.ap[0], [0, n], ap.ap[1]])


def build_nc():
    nc = bacc.Bacc("TRN2", target_bir_lowering=False, debug=False)

    # ---------------- DRAM tensors (per-core inputs) ----------------
    wi_d = nc.dram_tensor("wi", [NPOS, 1], I32, kind="ExternalInput")
    au_d = nc.dram_tensor("au", [NPOS, AUDIO], F32, kind="ExternalInput")
    lute_d = nc.dram_tensor("lute", [VOCAB, EMBD], F32, kind="ExternalInput")
    # lutp is [phase_tab0 | phase_tab1] concatenated host-side
    lutp_d = nc.dram_tensor("lutp", [VOCAB, D1 + D2], F32, kind="ExternalInput")
    pack0_d = nc.dram_tensor("pack0", [128, PACK0_COLS], F32, kind="ExternalInput")
    pack1_d = nc.dram_tensor("pack1", [128, 1536], BF16, kind="ExternalInput")
    pack2_d = nc.dram_tensor("pack2", [128, PACK2_COLS], F32, kind="ExternalInput")
    packb_d = nc.dram_tensor("packb", [128, PACKB_COLS], BF16, kind="ExternalInput")
    blin_d = nc.dram_tensor("blin", [1, D1], F32, kind="ExternalInput")
    mw_d = nc.dram_tensor("mw", [1, 2], F32, kind="ExternalInput")
    y_d = nc.dram_tensor("y", [BL, 1], F32, kind="ExternalOutput")

    with tile.TileContext(nc) as tc, ExitStack() as ctx:
        cpool = ctx.enter_context(tc.tile_pool(name="const", bufs=1))
        wpool = ctx.enter_context(tc.tile_pool(name="work", bufs=2))
        npool = ctx.enter_context(tc.tile_pool(name="newton", bufs=2))
        lpool = ctx.enter_context(tc.tile_pool(name="lstm", bufs=3))
        # PSUM budget is 8 banks; share one tag per pool so slots are reused.
        ps_g = ctx.enter_context(tc.tile_pool(name="psg", bufs=2, space="PSUM"))
        ps_a = ctx.enter_context(tc.tile_pool(name="psa", bufs=3, space="PSUM"))
        ps_b = ctx.enter_context(tc.tile_pool(name="psb", bufs=3, space="PSUM"))

        # ---------------- bulk loads (criticality order) ----------------
        # wi goes on the DVE queue so pack1 (which gates the GX matmuls) is
        # the first SP issue.
        # wi first on the gpsimd queue (ahead of its memsets) so the gather's
        # index data is the very first DMA issued.
        wi = cpool.tile([NPOS, 1], I32, tag="wi")
        nc.gpsimd.dma_start(wi[:], wi_d[:])
        pack1 = cpool.tile([128, 1536], BF16, tag="pack1")
        nc.sync.dma_start(pack1[:], pack1_d[:])
        packb = cpool.tile([128, PACKB_COLS], BF16, tag="packb")
        nc.sync.dma_start(packb[:], packb_d[:])
        pack0 = cpool.tile([128, PACK0_COLS], F32, tag="pack0")
        nc.sync.dma_start(pack0[:], pack0_d[:])
        pack2 = cpool.tile([128, PACK2_COLS], F32, tag="pack2")
        nc.sync.dma_start(pack2[:], pack2_d[:])

        def p0(name, nrow=128):
            c0, ncol = PACK0[name]
            return pack0[0:nrow, c0 : c0 + ncol]

        def p2(name, nrow=128):
            c0, ncol = PACK2[name]
            return pack2[0:nrow, c0 : c0 + ncol]

        def pb(name, nrow=128):
            c0, ncol = PACKB[name]
            return packb[0:nrow, c0 : c0 + ncol]

        wihta = pack1[:, 0:512]
        wihtb = pack1[:, 512:1024]
        wihtc = pack1[0:45, 1024:1536]
        whht = pb("whht")
        identb = pb("identb")
        wlinb = pb("wlinb")
        s1tb = pb("s1tb")
        s2tb = pb("s2tb")
        mr = p2("mr")
        mi = p2("mi")
        ident = p0("ident")
        fw1t = p0("fw1t")
        fw2t = p0("fw2t", 64)
        w1b = p0("w1b", 75)
        w2b = p0("w2b", 17)
        w3b = p0("w3b", 17)
        fb1 = p0("fb1", 64)
        fb2 = p0("fb2", 64)
        fw3t = p0("fw3t", 64)
        fb3 = p0("fb3", 1)
        au = cpool.tile([NPOS, AUDIO], F32, tag="au")
        nc.sync.dma_start(au[:], au_d[:])

        # partition-broadcast loads
        blin_bc = cpool.tile([128, D1], F32, tag="blin_bc")
        bap = blin_d[:]
        nc.sync.dma_start(
            blin_bc[:],
            bass.AP(tensor=bap.tensor, offset=bap.offset, ap=[[0, 128], bap.ap[1]]),
        )
        mwb = cpool.tile([128, 2], F32, tag="mwb")
        map_ = mw_d[:]
        nc.sync.dma_start(
            mwb[:],
            bass.AP(tensor=map_.tensor, offset=map_.offset, ap=[[0, 128], map_.ap[1]]),
        )

        # ---------------- gathers: emb first, then phases ------
        # emb gathers straight to bf16 via the casting SWDGE path
        emb = cpool.tile([NPOS, EMBD], BF16, tag="embg")
        nc.gpsimd.indirect_dma_start(
            out=emb[:], out_offset=None, in_=lute_d[:],
            in_offset=bass.IndirectOffsetOnAxis(ap=wi[:, 0:1], axis=0),
        )
        gathp = cpool.tile([NPOS, D1 + D2], F32, tag="gathp")
        nc.gpsimd.indirect_dma_start(
            out=gathp[:], out_offset=None, in_=lutp_d[:],
            in_offset=bass.IndirectOffsetOnAxis(ap=wi[:, 0:1], axis=0),
        )

        # ---------------- trig (ACT set #1) ----------------
        # HW Sin domain is [-pi, pi]: wrap both phase tabs together (3 passes
        # cover +-7pi), one extra shifted wrap gives the cos argument.
        PI = float(np.pi)
        TWO_PI = float(2 * np.pi)

        def wrap3(src_ap, width, tag):
            cur = src_ap
            for p in range(3):
                t = wpool.tile([128, width], F32, tag=f"{tag}_w")
                nc.vector.add_range_wrap(
                    out=t[:], in_=cur, shift=0.0, bound=PI, period=TWO_PI
                )
                cur = t[:]
            return cur

        phw = wrap3(gathp[:, 0 : D1 + D2], D1 + D2, "wr")
        phc = wpool.tile([128, D1 + D2], F32, tag="ca")
        nc.vector.add_range_wrap(
            out=phc[:], in_=phw, shift=HALF_PI, bound=PI, period=TWO_PI
        )
        sinall = cpool.tile([NPOS, D1 + D2], F32, tag="sinall")
        nc.scalar.activation(out=sinall[:], in_=phw, func=AF.Sin)
        cosall = cpool.tile([NPOS, D1 + D2], F32, tag="cosall")
        nc.scalar.activation(out=cosall[:], in_=phc[:], func=AF.Sin)
        sin0 = sinall[:, 0:D1]
        sin1 = sinall[:, D1 : D1 + D2]
        cos0 = cosall[:, 0:D1]
        cos1 = cosall[:, D1 : D1 + D2]

        # dummy exp-set trigger -- forced AFTER both Sin ops (ACT table
        # order), so the exp_and_others table load happens off the LSTM path.
        jj = wpool.tile([128, 1], F32, tag="jj")
        nc.vector.scalar_tensor_tensor(
            out=jj[:], in0=sinall[:, 0:1], scalar=0.0, in1=cosall[:, 0:1],
            op0=OP.mult, op1=OP.mult,
        )
        # ---- from here on, every ACT op uses exp_and_others (exp/tanh/square)
        dxp = wpool.tile([128, 1], F32, tag="dxp")
        nc.scalar.activation(out=dxp[:], in_=jj[:], func=AF.Exp)
        # modality softmax is host-precomputed into mwb
        mw0 = mwb[:, 0:1]
        mw1 = mwb[:, 1:2]

        # ---------------- embedding transpose + GX matmuls ----------------
        embT0 = cpool.tile([128, NPOS], BF16, tag="embT0")
        pst = ps_b.tile([128, 128], BF16, tag="psb")
        nc.tensor.transpose(pst[:], emb[:, 0:128], identb)
        nc.vector.tensor_copy(out=embT0[:], in_=pst[:])
        embT1 = cpool.tile([128, NPOS], BF16, tag="embT1")
        pst = ps_b.tile([128, 128], BF16, tag="psb")
        nc.tensor.transpose(pst[:], emb[:, 128:256], identb)
        nc.vector.tensor_copy(out=embT1[:], in_=pst[:])
        embT2 = cpool.tile([64, NPOS], BF16, tag="embT2")
        nc.vector.memset(embT2[:], 1.0)
        pst44 = ps_b.tile([44, 128], BF16, tag="psb")
        nc.tensor.transpose(pst44[:], emb[:, 256:300], identb)
        nc.vector.tensor_copy(out=embT2[0:44, :], in_=pst44[:])

        # GXT[hid, gate, pos] holds pre-activation z' (sigma-fold pre-scaled).
        # Two position blocks so early LSTM steps start before all GX is done.
        GXT = cpool.tile([HID, 4, NPOS], BF16, tag="GXT")
        for blk, (p0, p1) in enumerate(((0, 32), (32, NPOS))):
            for g in range(4):
                gsl = slice(HID * g, HID * (g + 1))
                pg = ps_a.tile([128, p1 - p0], F32, tag="psa")
                nc.tensor.matmul(
                    pg[:], wihta[:, gsl], embT0[:, p0:p1], start=True, stop=False
                )
                nc.tensor.matmul(
                    pg[:], wihtb[:, gsl], embT1[:, p0:p1], start=False, stop=False
                )
                nc.tensor.matmul(
                    pg[:], wihtc[:, gsl], embT2[0:45, p0:p1], start=False, stop=True
                )
                nc.vector.tensor_copy(out=GXT[:, g, p0:p1], in_=pg[:])

        # ---------------- audio MLP (feature-major) ----------------
        auT = cpool.tile([96, NPOS], F32, tag="auT")
        nc.vector.memset(auT[:], 1.0)
        psau = ps_b.tile([AUDIO, NPOS], F32, tag="psb")
        nc.tensor.transpose(psau[:], au[:], ident)
        nc.vector.tensor_copy(out=auT[0:AUDIO, :], in_=psau[:])

        a1b = cpool.tile([32, NPOS], F32, tag="a1b")
        nc.vector.memset(a1b[:], 1.0)
        psm = ps_b.tile([D2, NPOS], F32, tag="psb")
        nc.tensor.matmul(psm[:], w1b, auT[0 : AUDIO + 1, :], start=True, stop=True)
        nc.vector.tensor_scalar(
            out=a1b[0:D2, :], in0=psm[:], scalar1=0.0, scalar2=None, op0=OP.max
        )
        a2b = cpool.tile([32, NPOS], F32, tag="a2b")
        nc.vector.memset(a2b[:], 1.0)
        psm = ps_b.tile([D2, NPOS], F32, tag="psb")
        nc.tensor.matmul(psm[:], w2b, a1b[0 : D2 + 1, :], start=True, stop=True)
        nc.vector.tensor_scalar(
            out=a2b[0:D2, :], in0=psm[:], scalar1=0.0, scalar2=None, op0=OP.max
        )
        amp1T = cpool.tile([D2, NPOS], F32, tag="amp1T")
        psm = ps_b.tile([D2, NPOS], F32, tag="psb")
        nc.tensor.matmul(psm[:], w3b, a2b[0 : D2 + 1, :], start=True, stop=True)
        nc.vector.tensor_scalar(
            out=amp1T[:], in0=psm[:], scalar1=0.0, scalar2=None, op0=OP.max
        )
        # transpose to position-major
        amp1 = cpool.tile([NPOS, D2], F32, tag="amp1")
        ps16 = ps_b.tile([128, D2], F32, tag="psb")
        nc.tensor.transpose(ps16[:], amp1T[:], ident[0:D2, 0:D2])
        nc.vector.tensor_copy(out=amp1[:], in_=ps16[:])

        # n1 (norm) via prescaled Newton; the normalization itself is factored
        junk16 = wpool.tile([128, D2], F32, tag="junk16")
        s1x = wpool.tile([128, 1], F32, tag="s1x")
        nc.vector.scalar_tensor_tensor(
            out=junk16[:], in0=amp1[:], scalar=64.0, in1=amp1[:],
            op0=OP.mult, op1=OP.mult, accum_out=s1x[:, 0:1],
        )
        r1n = _newton_rsqrt(nc, npool, s1x[:], iters=3, tag="n1")
        n1 = wpool.tile([128, 1], F32, tag="n1v")
        nc.vector.tensor_scalar(
            out=n1[:], in0=s1x[:], scalar1=r1n[:, 0:1], scalar2=0.125,
            op0=OP.mult, op1=OP.mult,
        )
        wpart = cpool.tile([128, 1], F32, tag="wpart")
        nc.vector.tensor_scalar(
            out=wpart[:], in0=n1[:], scalar1=mw1[:, 0:1], scalar2=None, op0=OP.mult
        )
        # UNnormalized branch-1 vectors (1/(n1+eps) factored out, applied at M)
        r1v = cpool.tile([NPOS, D2], F32, tag="r1v")
        nc.vector.tensor_tensor(out=r1v[:], in0=amp1[:], in1=cos1[:], op=OP.mult)
        i1v = cpool.tile([NPOS, D2], F32, tag="i1v")
        nc.vector.tensor_tensor(out=i1v[:], in0=amp1[:], in1=sin1[:], op=OP.mult)
        uv = cpool.tile([NPOS, D2], F32, tag="uv")
        nc.vector.tensor_tensor(out=uv[:], in0=r1v[:], in1=i1v[:], op=OP.subtract)
        vv = cpool.tile([NPOS, D2], F32, tag="vv")
        nc.vector.tensor_tensor(out=vv[:], in0=r1v[:], in1=i1v[:], op=OP.add)

        # Force LSTM step-0 (first exp-set ACT op) after all four Sin outputs
        # so the trig table set is loaded exactly once, before exp_and_others.
        tg_a = wpool.tile([128, BL], F32, tag="tg_a")
        nc.vector.scalar_tensor_tensor(
            out=tg_a[:], in0=cos0[:, 0:BL], scalar=0.0, in1=sin0[:, 0:BL],
            op0=OP.mult, op1=OP.mult,
        )
        tg_b = wpool.tile([128, BL], F32, tag="tg_b")
        nc.vector.scalar_tensor_tensor(
            out=tg_b[:], in0=cos1[:, 0:BL], scalar=0.0, in1=sin1[:, 0:BL],
            op0=OP.mult, op1=OP.mult,
        )
        # scalar=dxp (==1) anchors the dummy-exp table load before step 0
        tg_c = wpool.tile([128, BL], BF16, tag="tg_c")
        nc.vector.scalar_tensor_tensor(
            out=tg_c[:], in0=tg_a[:], scalar=dxp[:, 0:1], in1=tg_b[:],
            op0=OP.mult, op1=OP.add,
        )
        nc.vector.scalar_tensor_tensor(
            out=GXT[:, 0, 0:BL], in0=tg_c[:], scalar=1.0, in1=GXT[:, 0, 0:BL],
            op0=OP.mult, op1=OP.add,
        )

        # ---------------- LSTM (hidden-major, doubled states) ----------------
        # bf16 h-state and recurrent weights: single-pass PE matmuls w/ FWL.
        H2 = cpool.tile([HID, NPOS], BF16, tag="H2")
        c2_prev = None
        for s in range(S):
            gx3 = GXT[:, :, s * BL : (s + 1) * BL]  # (128, 4, 4)
            gates = lpool.tile([128, 4, BL], F32, tag="gates")
            if s == 0:
                nc.scalar.activation(out=gates[:], in_=gx3, func=AF.Tanh)
            else:
                gp = ps_g.tile([128, 4, BL], F32, tag="gp")
                nc.tensor.matmul(gp[:], identb, gx3, start=True, stop=False)
                hprev = H2[:, (s - 1) * BL : s * BL]
                for g in range(4):
                    nc.tensor.matmul(
                        gp[:, g, :], whht[:, HID * g : HID * (g + 1)], hprev,
                        start=False, stop=(g == 3),
                    )
                nc.scalar.activation(out=gates[:], in_=gp[:], func=AF.Tanh)
            ti, tf = gates[:, 0, :], gates[:, 1, :]
            to, tg = gates[:, 2, :], gates[:, 3, :]
            q2 = lpool.tile([128, BL], F32, tag="q2")
            nc.vector.scalar_tensor_tensor(
                out=q2[:], in0=ti, scalar=1.0, in1=tg, op0=OP.add, op1=OP.mult
            )
            if s == 0:
                c2 = q2
            else:
                q1 = lpool.tile([128, BL], F32, tag="q1")
                nc.vector.scalar_tensor_tensor(
                    out=q1[:], in0=tf, scalar=1.0, in1=c2_prev[:],
                    op0=OP.add, op1=OP.mult,
                )
                c2 = lpool.tile([128, BL], F32, tag="c2")
                nc.vector.scalar_tensor_tensor(
                    out=c2[:], in0=q1[:], scalar=0.5, in1=q2[:],
                    op0=OP.mult, op1=OP.add,
                )
            th = lpool.tile([128, BL], F32, tag="th")
            nc.scalar.activation(out=th[:], in_=c2[:], func=AF.Tanh, scale=0.5)
            nc.vector.scalar_tensor_tensor(
                out=H2[:, s * BL : (s + 1) * BL], in0=to, scalar=1.0, in1=th[:],
                op0=OP.add, op1=OP.mult,
            )
            c2_prev = c2

        # measurement-kernel norm is folded into fw1t host-side
        # measurement tiles arranged so (P+T) and (R-Q) accumulate in PSUM:
        # vrviA_k = [vrT_k | -viT_k] (moving for real chunks),
        # vrviB_k = [viT_k |  vrT_k] (moving for imag chunks)
        vrviA = []
        vrviB = []
        for k in range(2):
            ta = cpool.tile([128, 2 * UNITS], BF16, tag=f"vrviA{k}")
            tb = cpool.tile([128, 2 * UNITS], BF16, tag=f"vrviB{k}")
            pvr = ps_b.tile([128, 128], F32, tag="psb")
            nc.tensor.transpose(pvr[:], mr[:, 128 * k : 128 * (k + 1)], ident)
            pvi = ps_b.tile([128, 128], F32, tag="psb")
            nc.tensor.transpose(pvi[:], mi[:, 128 * k : 128 * (k + 1)], ident)
            nc.scalar.copy(out=ta[:, 0:128], in_=pvr[:])
            nc.vector.tensor_scalar(
                out=ta[:, 128:256], in0=pvi[:], scalar1=-1.0, scalar2=None,
                op0=OP.mult,
            )
            nc.scalar.copy(out=tb[:, 0:128], in_=pvi[:])
            nc.vector.tensor_copy(out=tb[:, 128:256], in_=pvr[:])
            vrviA.append(ta)
            vrviB.append(tb)

        # ---------------- amp0 / weight path ----------------
        amp0p = ps_b.tile([NPOS, D1], F32, tag="psb")
        nc.tensor.matmul(amp0p[:], H2[:], wlinb, start=True, stop=True)
        amp0 = cpool.tile([NPOS, D1], F32, tag="amp0")
        nc.vector.tensor_tensor(out=amp0[:], in0=amp0p[:], in1=blin_bc[:], op=OP.add)
        junk16b = wpool.tile([128, D1], F32, tag="junk16b")
        s0x = wpool.tile([128, 1], F32, tag="s0x")
        nc.vector.scalar_tensor_tensor(
            out=junk16b[:], in0=amp0[:], scalar=8.0, in1=amp0[:],
            op0=OP.mult, op1=OP.mult, accum_out=s0x[:, 0:1],
        )
        # sqrt-free measurement scale: csq = 1/(s0*s1) = 512/(s0x*s1x)
        csp = wpool.tile([128, 1], F32, tag="csp")
        nc.vector.tensor_scalar(
            out=csp[:], in0=s0x[:], scalar1=s1x[:, 0:1], scalar2=1.0 / 512.0,
            op0=OP.mult, op1=OP.mult,
        )
        csq = cpool.tile([128, 1], F32, tag="csq")
        nc.vector.reciprocal(out=csq[:], in_=csp[:])
        # ---------------- tensor product -> realim (pos-major) ----------------
        # UNnormalized: the 1/(n0+eps), 1/(n1+eps) factors are applied at M.
        r0v = wpool.tile([NPOS, D1], F32, tag="r0v")
        nc.vector.tensor_tensor(out=r0v[:], in0=amp0[:], in1=cos0[:], op=OP.mult)
        i0v = wpool.tile([NPOS, D1], F32, tag="i0v")
        nc.vector.tensor_tensor(out=i0v[:], in0=amp0[:], in1=sin0[:], op=OP.mult)

        realim = cpool.tile([NPOS, 2 * DIM], BF16, tag="realim")
        tmpA = wpool.tile([NPOS, DIM], F32, tag="tmpA")
        tmpB = wpool.tile([NPOS, DIM], F32, tag="tmpB")
        # real = r0 (x) u - i0 (x) v   ; imag = r0 (x) v + i0 (x) u
        nc.vector.tensor_tensor(
            out=tmpA[:].rearrange("p (i j) -> p i j", j=D2),
            in0=r0v[:].to_broadcast([NPOS, D1, D2]),
            in1=_outer_bcast(uv[:], D1), op=OP.mult,
        )
        nc.vector.tensor_tensor(
            out=tmpB[:].rearrange("p (i j) -> p i j", j=D2),
            in0=i0v[:].to_broadcast([NPOS, D1, D2]),
            in1=_outer_bcast(vv[:], D1), op=OP.mult,
        )
        nc.vector.tensor_tensor(
            out=realim[:, 0:DIM], in0=tmpA[:], in1=tmpB[:], op=OP.subtract
        )
        tmpC = wpool.tile([NPOS, DIM], F32, tag="tmpC")
        tmpD = wpool.tile([NPOS, DIM], F32, tag="tmpD")
        nc.vector.tensor_tensor(
            out=tmpC[:].rearrange("p (i j) -> p i j", j=D2),
            in0=r0v[:].to_broadcast([NPOS, D1, D2]),
            in1=_outer_bcast(vv[:], D1), op=OP.mult,
        )
        nc.vector.tensor_tensor(
            out=tmpD[:].rearrange("p (i j) -> p i j", j=D2),
            in0=i0v[:].to_broadcast([NPOS, D1, D2]),
            in1=_outer_bcast(uv[:], D1), op=OP.mult,
        )
        nc.vector.tensor_tensor(
            out=realim[:, DIM : 2 * DIM], in0=tmpC[:], in1=tmpD[:], op=OP.add
        )

        # transpose realim -> rimT (dim-major), 4 chunks of 128
        rimT = cpool.tile([128, 4 * 128], BF16, tag="rimT")
        for q in range(4):
            pst = ps_b.tile([128, 128], BF16, tag="psb")
            nc.tensor.transpose(pst[:], realim[:, 128 * q : 128 * (q + 1)], identb)
            nc.scalar.copy(out=rimT[:, 128 * q : 128 * (q + 1)], in_=pst[:])

        # [P+T | R-Q] accumulates across all four matmuls in one PSUM tile
        ps_u = ps_a.tile([NPOS, 2 * UNITS], F32, tag="psa")
        nc.tensor.matmul(
            ps_u[:], rimT[:, 0:128], vrviA[0][:], start=True, stop=False
        )
        nc.tensor.matmul(
            ps_u[:], rimT[:, 128:256], vrviA[1][:], start=False, stop=False
        )
        nc.tensor.matmul(
            ps_u[:], rimT[:, 256:384], vrviB[0][:], start=False, stop=False
        )
        nc.tensor.matmul(
            ps_u[:], rimT[:, 384:512], vrviB[1][:], start=False, stop=True
        )
        # weight-path newton on DVE, emitted after the real-half: runs while
        # the PE does the rimT transposes + PQRT matmuls
        r0n = _newton_rsqrt_q(nc, npool, s0x[:], iters=3, tag="n0")
        n0 = wpool.tile([128, 1], F32, tag="n0v")
        nc.vector.tensor_scalar(
            out=n0[:], in0=s0x[:], scalar1=r0n[:, 0:1], scalar2=INV_SQRT8,
            op0=OP.mult, op1=OP.mult,
        )
        weight = wpool.tile([128, 1], F32, tag="weight")
        nc.vector.scalar_tensor_tensor(
            out=weight[:], in0=n0[:], scalar=mw0[:, 0:1], in1=wpart[:],
            op0=OP.mult, op1=OP.add,
        )
        wb16 = wpool.tile([128, 1], BF16, tag="wb16")
        nc.vector.tensor_copy(out=wb16[:], in_=weight[:])
        sq1 = wpool.tile([NPOS, UNITS], F32, tag="sq1")
        nc.scalar.activation(out=sq1[:], in_=ps_u[:, 0:UNITS], func=AF.Square)
        sq2 = wpool.tile([NPOS, UNITS], F32, tag="sq2")
        nc.scalar.activation(
            out=sq2[:], in_=ps_u[:, UNITS : 2 * UNITS], func=AF.Square
        )
        msu = wpool.tile([NPOS, UNITS], F32, tag="msu")
        nc.vector.tensor_tensor(out=msu[:], in0=sq1[:], in1=sq2[:], op=OP.add)
        msr = cpool.tile([NPOS, UNITS], BF16, tag="msr")
        nc.vector.tensor_scalar(
            out=msr[:], in0=msu[:], scalar1=csq[:, 0:1], scalar2=None,
            op0=OP.mult,
        )

        # windowed weights [sh4(w), sh8(w), w] assembled in PSUM, then softmax
        vp = ps_b.tile([128, 3], F32, tag="psb")
        nc.tensor.matmul(vp[:, 0:1], s1tb, wb16[:], start=True, stop=True)
        nc.tensor.matmul(vp[:, 1:2], s2tb, wb16[:], start=True, stop=True)
        nc.tensor.matmul(vp[:, 2:3], identb, wb16[:], start=True, stop=True)
        e3 = wpool.tile([128, 3], F32, tag="e3")
        esum = wpool.tile([128, 1], F32, tag="esum")
        nc.scalar.activation(
            out=e3[:], in_=vp[:], func=AF.Exp, accum_out=esum[:, 0:1]
        )
        res = wpool.tile([128, 1], F32, tag="res")
        nc.vector.reciprocal(out=res[:], in_=esum[:])
        ww = wpool.tile([128, 3], F32, tag="ww")
        nc.vector.tensor_scalar(
            out=ww[:], in0=e3[:], scalar1=res[:, 0:1], scalar2=None, op0=OP.mult
        )

        # n-gram mixing: m3 = ww0*M + ww1*shift4(M) + ww2*shift8(M)
        ps_ms = ps_a.tile([NPOS, 2 * UNITS], F32, tag="psa")
        nc.tensor.matmul(ps_ms[:, 0:UNITS], s1tb, msr[:], start=True, stop=True)
        nc.tensor.matmul(
            ps_ms[:, UNITS : 2 * UNITS], s2tb, msr[:], start=True, stop=True
        )
        t1m = wpool.tile([NPOS, UNITS], F32, tag="t1m")
        nc.vector.tensor_scalar(
            out=t1m[:], in0=ps_ms[:, 0:UNITS], scalar1=ww[:, 0:1], scalar2=None,
            op0=OP.mult,
        )
        m3a = wpool.tile([NPOS, UNITS], F32, tag="m3a")
        nc.vector.scalar_tensor_tensor(
            out=m3a[:], in0=ps_ms[:, UNITS : 2 * UNITS], scalar=ww[:, 1:2], in1=t1m[:],
            op0=OP.mult, op1=OP.add,
        )
        m3 = wpool.tile([NPOS, UNITS], F32, tag="m3")
        nc.vector.scalar_tensor_tensor(
            out=m3[:], in0=msr[:], scalar=ww[:, 2:3], in1=m3a[:],
            op0=OP.mult, op1=OP.add,
        )
        mmx = wpool.tile([NPOS, UNITS], BF16, tag="mmx")
        nc.vector.tensor_tensor(out=mmx[:], in0=msr[:], in1=m3[:], op=OP.max)

        # max over positions per batch: transpose then reduce over s
        ps_mt = ps_b.tile([UNITS, NPOS], BF16, tag="psb")
        nc.tensor.transpose(ps_mt[:], mmx[:], identb)
        featU = wpool.tile([UNITS, BL], F32, tag="featU")
        nc.vector.tensor_reduce(
            out=featU[:], in_=ps_mt[:].rearrange("p (s b) -> p b s", b=BL),
            axis=mybir.AxisListType.X, op=OP.max,
        )

        # ---------------- final MLP (fw1t pre-scaled by 1/kn^2) ----------------
        y1p = ps_b.tile([CELL, BL], F32, tag="psb")
        nc.tensor.matmul(y1p[:], fw1t, featU[:], start=True, stop=True)
        y1 = wpool.tile([CELL, BL], F32, tag="y1")
        nc.vector.tensor_scalar(
            out=y1[:], in0=y1p[:], scalar1=fb1[:, 0:1], scalar2=0.0,
            op0=OP.add, op1=OP.max,
        )
        y2p = ps_b.tile([CELL, BL], F32, tag="psb")
        nc.tensor.matmul(y2p[:], fw2t, y1[:], start=True, stop=True)
        y2 = wpool.tile([CELL, BL], F32, tag="y2")
        nc.vector.tensor_scalar(
            out=y2[:], in0=y2p[:], scalar1=fb2[:, 0:1], scalar2=0.0,
            op0=OP.add, op1=OP.max,
        )
        y3p = ps_b.tile([1, BL], F32, tag="psb")
        nc.tensor.matmul(y3p[:], fw3t, y2[:], start=True, stop=True)
        ysb = wpool.tile([1, BL], F32, tag="ysb")
        nc.vector.tensor_scalar(
            out=ysb[:], in0=y3p[:], scalar1=fb3[0:1, 0:1], scalar2=None, op0=OP.add
        )
        nc.sync.dma_start(y_d[:], ysb[:])

    nc.compile()
    return nc


_NC = None


def _get_nc():
    global _NC
    if _NC is None:
        _NC = build_nc()
    return _NC


def _padrows(a, rows):
    out = np.zeros((rows, a.shape[1]), np.float32)
    out[: a.shape[0]] = a
    return out


def make_in_maps(inputs):
    """Host-side layout prep (value-preserving transforms only)."""
    import ml_dtypes

    f32 = np.float32
    bf16 = ml_dtypes.bfloat16
    wi_full = np.asarray(inputs["word_indexes"]).astype(np.int32)  # (B, S)
    au_full = np.asarray(inputs["audio"], dtype=f32)  # (B, S, A)
    lute = np.ascontiguousarray(np.asarray(inputs["lookup_table"], dtype=f32))
    lutp = np.ascontiguousarray(
        np.concatenate(
            [
                np.asarray(inputs["phase_tab0"], dtype=f32),
                np.asarray(inputs["phase_tab1"], dtype=f32),
            ],
            axis=1,
        )
    )
    w_ih = np.asarray(inputs["w_ih"], dtype=f32)
    w_hh = np.asarray(inputs["w_hh"], dtype=f32)
    b_ih = np.asarray(inputs["b_ih"], dtype=f32)
    b_hh = np.asarray(inputs["b_hh"], dtype=f32)
    w_lin = np.asarray(inputs["w_lin"], dtype=f32)
    b_lin = np.asarray(inputs["b_lin"], dtype=f32)

    # gate reorder [i, f, o, g] + sigma->tanh fold (x0.5 on i,f,o rows);
    # extra x0.5 on all w_hh entries for the doubled hidden state H2=2h.
    perm = np.concatenate(
        [np.arange(0, 128), np.arange(128, 256), np.arange(384, 512),
         np.arange(256, 384)]
    )
    gsc = np.concatenate([np.full(384, 0.5, f32), np.ones(128, f32)])
    w_ih_p = w_ih[perm] * gsc[:, None]
    b_p = (b_ih + b_hh)[perm] * gsc
    wihT = np.concatenate([w_ih_p.T, b_p[None, :]], axis=0).astype(f32)  # (301,512)
    whht = (w_hh[perm] * gsc[:, None] * 0.5).T.astype(f32)  # (128, 512)

    pack1 = np.concatenate(
        [wihT[0:128], wihT[128:256], _padrows(wihT[256:301], 128)], axis=1
    )
    pack1 = np.ascontiguousarray(pack1.astype(bf16))

    w1 = np.asarray(inputs["w1"], dtype=f32)
    w2 = np.asarray(inputs["w2"], dtype=f32)
    w3 = np.asarray(inputs["w3"], dtype=f32)
    b1 = np.asarray(inputs["b1"], dtype=f32)
    b2 = np.asarray(inputs["b2"], dtype=f32)
    b3 = np.asarray(inputs["b3"], dtype=f32)
    mr_f = np.asarray(inputs["meas_r"], dtype=np.float64)
    mi_f = np.asarray(inputs["meas_i"], dtype=np.float64)
    kn_f = np.sqrt((mr_f**2 + mi_f**2).sum(-1)) + 1e-10  # per-unit norm
    rkn2_f = (1.0 / kn_f**2).astype(f32)  # (UNITS,)
    parts0 = {
        "ident": np.eye(128, dtype=f32),
        "fw1t": np.asarray(inputs["fw1"], dtype=f32).T * rkn2_f[:, None],
        "fw2t": np.asarray(inputs["fw2"], dtype=f32).T,
        "w1b": np.concatenate([w1.T, b1[None, :]], 0),
        "w2b": np.concatenate([w2.T, b2[None, :]], 0),
        "w3b": np.concatenate([w3.T, b3[None, :]], 0),
        "fb1": np.asarray(inputs["fb1"], dtype=f32).reshape(CELL, 1),
        "fb2": np.asarray(inputs["fb2"], dtype=f32).reshape(CELL, 1),
        "fw3t": np.asarray(inputs["fw3"], dtype=f32).T.reshape(CELL, 1),
        "fb3": np.asarray(inputs["fb3"], dtype=f32).reshape(1, 1),
    }
    pack0 = np.zeros((128, PACK0_COLS), f32)
    for name, (c0, ncol) in PACK0.items():
        arr = np.asarray(parts0[name], dtype=f32)
        assert arr.shape[1] == ncol and arr.shape[0] <= 128, (name, arr.shape)
        pack0[: arr.shape[0], c0 : c0 + ncol] = arr
    pack0 = np.ascontiguousarray(pack0)
    parts2 = {
        "mr": np.asarray(inputs["meas_r"], dtype=f32),
        "mi": np.asarray(inputs["meas_i"], dtype=f32),
    }
    pack2 = np.zeros((128, PACK2_COLS), f32)
    for name, (c0, ncol) in PACK2.items():
        arr = np.asarray(parts2[name], dtype=f32)
        pack2[: arr.shape[0], c0 : c0 + ncol] = arr
    pack2 = np.ascontiguousarray(pack2)

    partsb = {
        "identb": np.eye(128, dtype=f32),
        "whht": whht,
        "wlinb": (0.5 * w_lin).T,
        "s1tb": np.eye(128, k=4, dtype=f32).T,
        "s2tb": np.eye(128, k=8, dtype=f32).T,
    }
    packb = np.zeros((128, PACKB_COLS), f32)
    for name, (c0, ncol) in PACKB.items():
        arr = np.asarray(partsb[name], dtype=f32)
        packb[: arr.shape[0], c0 : c0 + ncol] = arr
    packb = np.ascontiguousarray(packb.astype(bf16))

    blin = np.ascontiguousarray(b_lin.reshape(1, D1), dtype=f32)
    mwraw = np.asarray(inputs["modality_weights"], dtype=np.float64).reshape(2)
    mwe = np.exp(mwraw - mwraw.max())
    mw = np.ascontiguousarray((mwe / mwe.sum()).reshape(1, 2).astype(f32))

    shared = dict(
        lute=lute, lutp=lutp, pack0=pack0, pack1=pack1, pack2=pack2,
        packb=packb, blin=blin, mw=mw,
    )
    in_maps = []
    for c in range(NCORES):
        bs = slice(BL * c, BL * (c + 1))
        # s-major position order: pos = s*BL + b
        wi_c = np.ascontiguousarray(wi_full[bs].T.reshape(NPOS, 1))
        au_c = np.ascontiguousarray(
            au_full[bs].transpose(1, 0, 2).reshape(NPOS, AUDIO)
        )
        m = dict(shared)
        m["wi"] = wi_c
        m["au"] = au_c
        in_maps.append(m)
    return in_maps


def kernel(**inputs):
    from concourse.bass_utils import run_bass_kernel_spmd

    nc = _get_nc()
    in_maps = make_in_maps(inputs)
    res = run_bass_kernel_spmd(nc, in_maps, core_ids=list(range(NCORES)))
    out = np.concatenate(
        [np.asarray(res.results[c]["y"]).reshape(BL, 1) for c in range(NCORES)], axis=0
    ).astype(np.float32)
    return out

